# revision 18
# baseline (speedup 1.0000x reference)
"""Trainium2 Bass kernel for the GRU-GCN cell (nn_GRUCell).

Sharding: 8 NeuronCores.
 - Attention phases (logits/softmax/PV) are BATCH-parallel: each core owns
   4 batches and all 1024 nodes, so matmul free dims are 512 wide and each
   (batch, n-chunk) block is one deep pipeline of 8 matmuls + 8 exps.
 - Weight-gen and the per-node output matmuls are NODE-parallel (128
   nodes/core, all 32 batches): the per-node weights are too large to
   replicate or gather.
 - Three AllToAll collectives redistribute xg2 (gate), z*state, and xg2
   (update) between the batch-parallel and node-parallel layouts.
 - exp(logits) is spilled to DRAM between the gate and update GCNs (the
   update GCN reuses the same softmax).
All matmuls fp16 operands with fp32 PSUM accumulation.
"""

import os
import sys

sys.path.insert(0, "/opt/trn_rl_repo")
import numpy as np

B, N, D = 32, 1024, 64
DI = DO = 64
C = DI + DO          # 128
OG, OU = 2 * DO, DO  # 128, 64
NCORES = 8
NL = N // NCORES     # 128 nodes per core
BL = B // NCORES     # 4 batches per core
NG = NL // 4         # 32 col-pack groups of 4 nodes
EPS = 1e-12

_CACHE = {}
LAST_RESULT = None  # test harness reads timing info from here


def _np_reference(x, state, node_emb, time_emb, gate_w, gate_b, gate_gamma,
                  gate_beta, upd_w, upd_b, upd_gamma, upd_beta):
    """Plain numpy fallback (general layernorm parameters)."""

    def _ln(v, g, b2):
        mu = v.mean(-1, keepdims=True)
        var = ((v - mu) ** 2).mean(-1, keepdims=True)
        return (v - mu) / np.sqrt(var + EPS) * g + b2

    def _gcn(xg, w_pool, b_pool, g, b2):
        emb = _ln(node_emb[None] + time_emb[:, None], g, b2)
        logits = np.einsum("bnd,bmd->bnm", emb, emb, optimize=True)
        a = np.exp(logits - logits.max(-1, keepdims=True))
        a /= a.sum(-1, keepdims=True)
        xg2 = np.einsum("bnm,bmc->bnc", a, xg, optimize=True)
        w = np.einsum("nd,dkio->nkio", node_emb, w_pool, optimize=True)
        bias = time_emb @ b_pool
        return (np.einsum("bni,nio->bno", xg, w[:, 0], optimize=True)
                + np.einsum("bni,nio->bno", xg2, w[:, 1], optimize=True)
                + bias[:, None, :])

    inp = np.concatenate([x, state], -1)
    zr = 1.0 / (1.0 + np.exp(-_gcn(inp, gate_w, gate_b, gate_gamma, gate_beta)))
    z, r = zr[..., :DO], zr[..., DO:]
    cand = np.concatenate([x, z * state], -1)
    hc = np.tanh(_gcn(cand, upd_w, upd_b, upd_gamma, upd_beta))
    return (r * state + (1.0 - r) * hc).astype(np.float32)


def _install_prof_shim():
    """Provide antenv.axon_hooks if absent so trace=True can NTFF-profile."""
    import types

    if "antenv.axon_hooks" in sys.modules:
        return
    try:
        from trn_agent_boot.trn_boot import _ntff_profile_via_ctypes

        hook = _ntff_profile_via_ctypes("/opt/axon/libaxon_pjrt.so")
    except Exception:
        hook = None
    mod = types.ModuleType("antenv.axon_hooks")
    mod.get_axon_ntff_profile_hook = lambda: hook

    def _set(h):
        mod.get_axon_ntff_profile_hook = lambda: h

    mod.set_axon_ntff_profile_hook = _set
    sys.modules["antenv.axon_hooks"] = mod
    try:
        import antenv

        antenv.axon_hooks = mod
    except Exception:
        pass


def _build():
    import concourse.bacc as bacc
    import concourse.mybir as mybir
    from concourse.tile import TileContext
    from concourse.masks import make_identity

    F16 = mybir.dt.float16
    F32 = mybir.dt.float32
    AF = mybir.ActivationFunctionType
    ALU = mybir.AluOpType

    nc = bacc.Bacc()

    def pin(name, shape, dt=F16):
        return nc.declare_dram_parameter(name, shape, dt, isOutput=False)

    neT_full = pin("neT_full", [D, N])          # node_emb^T, all nodes (LN)
    neT_loc2 = pin("neT_loc2", [128, NL])       # local node_emb^T, k-duplicated
    te_col = pin("te_col", [D, BL], F32)             # local time_emb columns
    teT16 = pin("teT16", [D, B])                # bias matmul lhsT
    gb16 = pin("gb16", [D, OG])
    ub16 = pin("ub16", [D, OU])
    inp_cm_h = pin("inp_cm_h", [128, BL * 8 * C])   # [m,(bb,q,c)] PV lhsT
    inpT_h = pin("inpT_h", [C, B * NL])         # c-major local [x;state]
    st_grp_h = pin("st_grp_h", [128, NG * DO])  # grouped local state
    pg_h = pin("pg_h", [128, OG * C])           # gate pool [64k+d,(o,i)]
    pu_h = pin("pu_h", [128, OU * C])           # upd pool
    h_out = nc.declare_dram_parameter("h_out", [128, NG * DO], F32, isOutput=True)

    with TileContext(nc) as tc:
        with (
            tc.tile_pool(name="const", bufs=1) as cpool,
            tc.tile_pool(name="big", bufs=1) as big,
            tc.tile_pool(name="stage", bufs=2) as stg,
            tc.tile_pool(name="dram", bufs=1, space="DRAM") as dram,
            tc.tile_pool(name="pswg", bufs=1, space="PSUM") as pswg,
        ):
            # ---------- constants ----------
            ones16r = cpool.tile([1, 128], F16, tag="ones16r")
            nc.gpsimd.memset(ones16r[:], 1.0)
            ones_col16 = cpool.tile([128, 1], F16, tag="ones_col16")
            nc.gpsimd.memset(ones_col16[:], 1.0)
            ones64 = cpool.tile([D, D], F16, tag="ones64")
            nc.gpsimd.memset(ones64[:], 1.0)
            neg64_col = cpool.tile([128, 1], F32, tag="neg64_col")
            nc.gpsimd.memset(neg64_col[:], -64.0)
            ident16 = cpool.tile([128, 128], F16, tag="ident16")
            make_identity(nc, ident16[:])
            eps_col = cpool.tile([D, 1], F32, tag="eps_col")
            nc.gpsimd.memset(eps_col[:], EPS)

            # ---------- persistent SBUF ----------
            neT_sb = cpool.tile([D, N], F16, tag="neT_sb")
            nc.sync.dma_start(neT_sb[:], neT_full[:])
            neL_sb = cpool.tile([128, NL], F16, tag="neL_sb")
            nc.sync.dma_start(neL_sb[:], neT_loc2[:])
            te_sb = cpool.tile([D, BL], F32, tag="te_sb")
            nc.sync.dma_start(te_sb[:], te_col[:])
            teT_sb = cpool.tile([D, B], F16, tag="teT_sb")
            nc.sync.dma_start(teT_sb[:], teT16[:])
            gb_sb = cpool.tile([D, OG], F16, tag="gb_sb")
            nc.sync.dma_start(gb_sb[:], gb16[:])
            ub_sb = cpool.tile([D, OU], F16, tag="ub_sb")
            nc.sync.dma_start(ub_sb[:], ub16[:])

            wslab = big.tile([C, 2 * OG * NL], F16, tag="wslab")      # 64KB/p
            zr_sb = big.tile([128, NG * OG], F16, tag="zr_sb")
            st_grp = big.tile([128, NG * DO], F16, tag="st_grp")
            nc.sync.dma_start(st_grp[:], st_grp_h[:])
            sinv16 = big.tile([D, BL * N], F16, tag="sinv16")
            s_all = big.tile([1, 8 * 512], F16, tag="s_all")
            sinv_rows = big.tile([1, 8 * 512], F16, tag="sinv_rows")
            bg16 = big.tile([B, OG], F16, tag="bg16")
            bu16 = big.tile([B, OU], F16, tag="bu16")

            # DRAM scratch: exp spill + three AllToAlls
            d_exp = dram.tile([BL * 2, 128, 8 * 512], F16, tag="d_exp")
            d_srow = dram.tile([8 * 512], F16, tag="d_srow")
            d_sinv = dram.tile([8 * 512], F16, tag="d_sinv")
            d1_in = dram.tile([NCORES, C, BL, NL], F16, tag="d1_in")
            d1_out = dram.tile([NCORES, C, BL, NL], F16, tag="d1_out")
            d2_in = dram.tile([NCORES, DO, BL, NL], F16, tag="d2_in")
            d2_out = dram.tile([NCORES, DO, BL, NL], F16, tag="d2_out")
            d3_in = dram.tile([NCORES, DO, BL, NL], F16, tag="d3_in")
            d3_out = dram.tile([NCORES, DO, BL, NL], F16, tag="d3_out")

            # ---------- bias = time_emb @ pool_b, replicated to (jj,b) rows --
            with tc.tile_pool(name="psb", bufs=1, space="PSUM") as psb:
                ps_bg = psb.tile([B, OG], F32, tag="ps_bg")
                nc.tensor.matmul(ps_bg[:], teT_sb[:], gb_sb[:], start=True, stop=True)
                nc.vector.tensor_copy(bg16[:], ps_bg[:])
                ps_bu = psb.tile([B, OU], F32, tag="ps_bu")
                nc.tensor.matmul(ps_bu[:], teT_sb[:], ub_sb[:], start=True, stop=True)
                nc.vector.tensor_copy(bu16[:], ps_bu[:])

            # ================= batch-parallel phase =================
            with tc.tile_pool(name="attn_sb", bufs=1) as asb:
                inp_cm = asb.tile([128, BL * 8 * C], F16, tag="inp_cm")
                nc.sync.dma_start(inp_cm[:], inp_cm_h[:])
                embT = asb.tile([D, BL * N], F16, tag="embT")
                xg2T_loc = asb.tile([C, BL * N], F16, tag="xg2T_loc")

                # ---- layernorm in transposed [d, n] layout, local batches ----
                with (
                    tc.tile_pool(name="ln_sb", bufs=1) as lsb,
                    tc.tile_pool(name="ln_ps", bufs=1, space="PSUM") as lps,
                ):
                    for bb in range(BL):
                        u16 = lsb.tile([D, N], F16, tag="u16")
                        nc.vector.tensor_scalar(
                            out=u16[:], in0=neT_sb[:],
                            scalar1=te_sb[:, bb: bb + 1],
                            scalar2=None, op0=ALU.add,
                        )
                        u2 = lsb.tile([D, N], F16, tag="u2")
                        nc.vector.tensor_mul(u2[:], u16[:], u16[:])
                        ps_sum = lps.tile([D, N], F32, tag="ps_sum")
                        ps_sq = lps.tile([D, N], F32, tag="ps_sq")
                        for hh in range(2):
                            sl = slice(hh * 512, (hh + 1) * 512)
                            nc.tensor.matmul(ps_sum[:, sl], ones64[:], u16[:, sl],
                                             start=True, stop=True)
                            nc.tensor.matmul(ps_sq[:, sl], ones64[:], u2[:, sl],
                                             start=True, stop=True)
                        mu16 = lsb.tile([D, N], F16, tag="mu16")
                        nc.vector.tensor_scalar_mul(mu16[:], ps_sum[:], 1.0 / D)
                        var32 = lsb.tile([D, N], F32, tag="var32")
                        nc.vector.tensor_scalar_mul(var32[:], ps_sq[:], 1.0 / D)
                        musq = lsb.tile([D, N], F32, tag="musq")
                        nc.vector.tensor_mul(musq[:], mu16[:], mu16[:])
                        nc.vector.tensor_sub(var32[:], var32[:], musq[:])
                        nc.scalar.activation(var32[:], var32[:], AF.Sqrt, bias=eps_col[:])
                        nc.vector.reciprocal(var32[:], var32[:])
                        dst = embT[:, bb * N: (bb + 1) * N]
                        nc.vector.tensor_sub(u16[:], u16[:], mu16[:])
                        nc.vector.tensor_mul(dst, u16[:], var32[:])

                # ---- attention: logits -> exp -> s -> PV, per (bb, ncol) ----
                with (
                    tc.tile_pool(name="psl", bufs=3, space="PSUM") as psl,
                    tc.tile_pool(name="psx", bufs=1, space="PSUM") as psx,
                    tc.tile_pool(name="pss", bufs=1, space="PSUM") as pss,
                ):
                    for bb in range(BL):
                        for ncol in range(2):
                            nb = ncol * 512
                            exp_w = stg.tile([128, 8 * 512], F16, tag="exp_w")
                            for q in range(8):
                                ps_l = psl.tile([128, 512], F32, tag="ps_l")
                                nc.tensor.matmul(
                                    ps_l[:],
                                    embT[:, bb * N + q * 128: bb * N + q * 128 + 128],
                                    embT[:, bb * N + nb: bb * N + nb + 512],
                                    start=True, stop=True,
                                )
                                nc.scalar.activation(
                                    exp_w[:, q * 512: (q + 1) * 512],
                                    ps_l[:], AF.Exp, bias=neg64_col[:],
                                )
                            nc.sync.dma_start(d_exp[bb * 2 + ncol], exp_w[:])
                            # denominator: ones-matmul accumulated over m-chunks
                            ps_s = pss.tile([1, 512], F32, tag="ps_s")
                            for q in range(8):
                                nc.tensor.matmul(
                                    ps_s[:], ones_col16[:],
                                    exp_w[:, q * 512: (q + 1) * 512],
                                    start=(q == 0), stop=(q == 7),
                                )
                            it = bb * 2 + ncol
                            nc.scalar.activation(
                                s_all[:, it * 512: (it + 1) * 512], ps_s[:],
                                AF.Copy)
                            # PV: xg2^T[c, n] accumulated over m-chunks
                            # (normalization deferred to the packed-recip pass)
                            ps_xg2 = psx.tile([C, 512], F32, tag="ps_xg2")
                            for q in range(8):
                                nc.tensor.matmul(
                                    ps_xg2[:],
                                    inp_cm[:, (bb * 8 + q) * C: (bb * 8 + q) * C + C],
                                    exp_w[:, q * 512: (q + 1) * 512],
                                    start=(q == 0), stop=(q == 7),
                                )
                            nc.vector.tensor_copy(
                                xg2T_loc[:, bb * N + nb: bb * N + nb + 512],
                                ps_xg2[:],
                            )

                # ---- packed softmax reciprocal, normalize, stage A2A#1 --
                with tc.tile_pool(name="psr", bufs=2, space="PSUM") as psr:
                    s_pack = stg.tile([128, 32], F16, tag="s_pack")
                    nc.gpsimd.dma_start(d_srow[:], s_all[:])
                    nc.gpsimd.dma_start(
                        s_pack[:], d_srow[:].rearrange("(j p) -> p j", p=128))
                    is_pack = stg.tile([128, 32], F16, tag="is_pack")
                    with nc.allow_low_precision("softmax 1/s in fp16"):
                        nc.vector.reciprocal(is_pack[:], s_pack[:])
                    nc.gpsimd.dma_start(
                        d_sinv[:].rearrange("(j p) -> p j", p=128), is_pack[:])
                    nc.gpsimd.dma_start(sinv_rows[:], d_sinv[:])
                    for bb in range(BL):
                        for ncol in range(2):
                            nb = ncol * 512
                            it = bb * 2 + ncol
                            ps_rep = psr.tile([128, 512], F32, tag="ps_rep")
                            nc.tensor.matmul(
                                ps_rep[:], ones16r[:],
                                sinv_rows[:, it * 512: (it + 1) * 512],
                                start=True, stop=True)
                            sl = slice(bb * N + nb, bb * N + nb + 512)
                            nc.vector.tensor_mul(
                                xg2T_loc[:, sl], xg2T_loc[:, sl], ps_rep[:])
                            nc.scalar.activation(
                                sinv16[:, sl], ps_rep[0:D, :], AF.Copy)
                            nc.gpsimd.dma_start(
                                d1_in[:].rearrange("j c bb nn -> c bb j nn")
                                [:, bb, 4 * ncol: 4 * ncol + 4, :],
                                xg2T_loc[:, sl].rearrange(
                                    "c (j nn) -> c j nn", nn=NL),
                            )

                # ---- w-gen gate pool (row-tiled k pairs) ----
                for och in range(8):
                    pwg_t = stg.tile([128, 16 * C], F16, tag="pwg_t")
                    nc.sync.dma_start(pwg_t[:], pg_h[:, och * 16 * C: (och + 1) * 16 * C])
                    for og4 in range(4):
                        ps_wA = pswg.tile([128, 4 * NL], F32, tag="wA")
                        ps_wB = pswg.tile([128, 4 * NL], F32, tag="wB")
                        for oo in range(4):
                            osl = slice((og4 * 4 + oo) * C, (og4 * 4 + oo) * C + C)
                            nc.tensor.matmul(
                                ps_wA[:, oo * NL: (oo + 1) * NL],
                                pwg_t[0:64, osl], neL_sb[0:64, :],
                                start=True, stop=True,
                            )
                            nc.tensor.matmul(
                                ps_wB[:, oo * NL: (oo + 1) * NL],
                                pwg_t[64:128, osl], neL_sb[64:128, :],
                                start=True, stop=True,
                            )
                        ob = (och * 4 + og4) * 4
                        nc.vector.tensor_copy(
                            wslab[:, ob * NL: (ob + 4) * NL], ps_wA[:])
                        nc.scalar.activation(
                            wslab[:, (OG + ob) * NL: (OG + ob + 4) * NL],
                            ps_wB[:], AF.Copy)

                # ---- AllToAll #1: xg2 gate, batch-shard -> node-shard ----

            nc.gpsimd.collective_compute(
                "AllToAll", mybir.AluOpType.bypass,
                replica_groups=[list(range(NCORES))],
                ins=[d1_in.opt()], outs=[d1_out.opt()],
            )

            # ================= node-parallel phase =================
            with tc.tile_pool(name="mid_sb", bufs=1) as msb:
                inpT_cm = msb.tile([C, B * NL], F16, tag="inpT_cm")
                nc.sync.dma_start(inpT_cm[:], inpT_h[:])
                xg2T_node = msb.tile([C, B * NL], F16, tag="xg2T_node")
                nc.sync.dma_start(
                    xg2T_node[:].rearrange("c (j r) -> c j r", j=NCORES),
                    d1_out[:].rearrange("j c bb nn -> c j (bb nn)"),
                )

                # ---- gate out-matmuls -> z, r ----
                wview = wslab[:].rearrange("c (k o n) -> c k o n", k=2, o=OG)
                with tc.tile_pool(name="pso", bufs=3, space="PSUM") as pso:
                    for g in range(NG):
                        ps_og = pso.tile([128, OG], F32, tag="og")
                        for jj in range(4):
                            n_ = g * 4 + jj
                            for ki, src in ((0, inpT_cm), (1, xg2T_node)):
                                lhs = src[:].rearrange("c (b n) -> c n b", n=NL)[:, n_, :]
                                rhs = wview[:, ki, :, n_]
                                nc.tensor.matmul(
                                    ps_og[32 * jj: 32 * jj + 32, :],
                                    lhs, rhs,
                                    start=(ki == 0), stop=False,
                                    tile_position=(0, 32 * jj),
                                )
                            nc.tensor.matmul(
                                ps_og[32 * jj: 32 * jj + 32, :],
                                ident16[0:B, 0:B], bg16[:],
                                start=False, stop=True,
                                tile_position=(0, 32 * jj),
                            )
                        nc.scalar.activation(
                            zr_sb[:, g * OG: (g + 1) * OG], ps_og[:], AF.Sigmoid)

                with tc.tile_pool(name="late_sb", bufs=1) as lsb2:
                    # keep the A@x channel block + x rows before mid_sb closes
                    xg2u_full = lsb2.tile([C, B * NL], F16, tag="xg2u_full")
                    nc.vector.tensor_copy(xg2u_full[0:DI, :], xg2T_node[0:DI, :])
                    candT = lsb2.tile([C, B * NL], F16, tag="candT")
                    nc.vector.tensor_copy(candT[0:DI, :], inpT_cm[0:DI, :])

                    zs_grp = lsb2.tile([128, NG * DO], F16, tag="zs_grp")
                    nc.vector.tensor_mul(
                        zs_grp[:].rearrange("p (g o) -> p g o", o=DO),
                        zr_sb[:].rearrange("p (g o) -> p g o", o=OG)[:, :, 0:DO],
                        st_grp[:].rearrange("p (g o) -> p g o", o=DO),
                    )

                    # candT rows 64:128 = (z*state)^T local (PE transpose)
                    with tc.tile_pool(name="psz", bufs=2, space="PSUM") as psz:
                        for g in range(NG):
                            ps_zt = psz.tile([DO, 128], F16, tag="ps_zt")
                            nc.tensor.transpose(
                                ps_zt[:], zs_grp[:, g * DO: (g + 1) * DO], ident16[:])
                            dst = (
                                candT[DI:C, :]
                                .rearrange("c (b n) -> c b n", n=NL)
                                [:, :, g * 4: g * 4 + 4]
                            )
                            src = ps_zt[:].rearrange("c (jj b) -> c b jj", jj=4)
                            nc.vector.tensor_copy(dst, src)

                    # ---- AllToAll #2: (z*state)^T, node-shard -> batch-shard
                    zsv = candT[DI:C, :].rearrange("o (b n) -> o b n", n=NL)
                    for j in range(NCORES):
                        nc.gpsimd.dma_start(
                            d2_in[j], zsv[:, BL * j: BL * j + BL, :])
                    nc.gpsimd.collective_compute(
                        "AllToAll", mybir.AluOpType.bypass,
                        replica_groups=[list(range(NCORES))],
                        ins=[d2_in.opt()], outs=[d2_out.opt()],
                    )

                    # ---- w-gen update pool (reuses wslab low half) ----
                    for och in range(4):
                        pwu_t = stg.tile([128, 16 * C], F16, tag="pwg_t")
                        nc.sync.dma_start(
                            pwu_t[:], pu_h[:, och * 16 * C: (och + 1) * 16 * C])
                        for og4 in range(4):
                            ps_wA = pswg.tile([128, 4 * NL], F32, tag="wA")
                            ps_wB = pswg.tile([128, 4 * NL], F32, tag="wB")
                            for oo in range(4):
                                osl = slice((og4 * 4 + oo) * C, (og4 * 4 + oo) * C + C)
                                nc.tensor.matmul(
                                    ps_wA[:, oo * NL: (oo + 1) * NL],
                                    pwu_t[0:64, osl], neL_sb[0:64, :],
                                    start=True, stop=True,
                                )
                                nc.tensor.matmul(
                                    ps_wB[:, oo * NL: (oo + 1) * NL],
                                    pwu_t[64:128, osl], neL_sb[64:128, :],
                                    start=True, stop=True,
                                )
                            ob = (och * 4 + og4) * 4
                            nc.vector.tensor_copy(
                                wslab[:, ob * NL: (ob + 4) * NL], ps_wA[:])
                            nc.scalar.activation(
                                wslab[:, (OU + ob) * NL: (OU + ob + 4) * NL],
                                ps_wB[:], AF.Copy)

                    zs_cmT = lsb2.tile([DO, BL * N], F16, tag="zs_cmT")
                    zcv = zs_cmT[:].rearrange("o (bb j nn) -> o bb j nn",
                                              j=NCORES, nn=NL)
                    for j in range(NCORES):
                        nc.sync.dma_start(zcv[:, :, j, :], d2_out[j])
                    zs_cm = lsb2.tile([128, BL * 8 * DO], F16, tag="zs_cm")
                    with tc.tile_pool(name="pszz", bufs=2, space="PSUM") as pszz:
                        for bb in range(BL):
                            for q in range(8):
                                ps_zc = pszz.tile([128, DO], F16, tag="ps_zc")
                                nc.tensor.transpose(
                                    ps_zc[:],
                                    zs_cmT[:, (bb * 8 + q) * NL:
                                           (bb * 8 + q) * NL + NL],
                                    ident16[0:DO, 0:DO],
                                )
                                nc.vector.tensor_copy(
                                    zs_cm[:, (bb * 8 + q) * DO:
                                          (bb * 8 + q) * DO + DO],
                                    ps_zc[:],
                                )

                    # ---- update PV (z*state part) from spilled exp ----
                    xg2u_loc = lsb2.tile([DO, BL * N], F16, tag="xg2u_loc")
                    with tc.tile_pool(name="psu", bufs=2, space="PSUM") as psu:
                        for bb in range(BL):
                            for ncol in range(2):
                                nb = ncol * 512
                                exp_r = stg.tile([128, 8 * 512], F16, tag="exp_w")
                                nc.sync.dma_start(exp_r[:], d_exp[bb * 2 + ncol])
                                ps_xu = psu.tile([DO, 512], F32, tag="ps_xu")
                                for q in range(8):
                                    nc.tensor.matmul(
                                        ps_xu[:],
                                        zs_cm[:, (bb * 8 + q) * DO:
                                              (bb * 8 + q) * DO + DO],
                                        exp_r[:, q * 512: (q + 1) * 512],
                                        start=(q == 0), stop=(q == 7),
                                    )
                                nc.vector.tensor_mul(
                                    xg2u_loc[:, bb * N + nb: bb * N + nb + 512],
                                    ps_xu[:],
                                    sinv16[:, bb * N + nb: bb * N + nb + 512],
                                )

                    # ---- AllToAll #3: xg2 upd, batch-shard -> node-shard ----
                    xg2uv = xg2u_loc[:].rearrange("c (bb j nn) -> c bb j nn",
                                                  j=NCORES, nn=NL)
                    for j in range(NCORES):
                        nc.gpsimd.dma_start(d3_in[j], xg2uv[:, :, j, :])
                    nc.gpsimd.collective_compute(
                        "AllToAll", mybir.AluOpType.bypass,
                        replica_groups=[list(range(NCORES))],
                        ins=[d3_in.opt()], outs=[d3_out.opt()],
                    )
                    nc.sync.dma_start(
                        xg2u_full[DI:C, :].rearrange("c (j r) -> c j r",
                                                     j=NCORES),
                        d3_out[:].rearrange("j c bb nn -> c j (bb nn)"),
                    )

                    # ---- update out-matmuls -> hc ----
                    hc_sb = lsb2.tile([128, NG * DO], F16, tag="hc_sb")
                    wuview = (
                        wslab[:, : 2 * OU * NL]
                        .rearrange("c (k o n) -> c k o n", k=2, o=OU)
                    )
                    with tc.tile_pool(name="psou", bufs=3, space="PSUM") as psou:
                        for g in range(NG):
                            ps_ou = psou.tile([128, OU], F32, tag="ou")
                            for jj in range(4):
                                n_ = g * 4 + jj
                                for ki, src in ((0, candT), (1, xg2u_full)):
                                    lhs = src[:].rearrange(
                                        "c (b n) -> c n b", n=NL)[:, n_, :]
                                    rhs = wuview[:, ki, :, n_]
                                    nc.tensor.matmul(
                                        ps_ou[32 * jj: 32 * jj + 32, :],
                                        lhs, rhs,
                                        start=(ki == 0), stop=False,
                                        tile_position=(0, 32 * jj),
                                    )
                                nc.tensor.matmul(
                                    ps_ou[32 * jj: 32 * jj + 32, :],
                                    ident16[0:B, 0:B], bu16[:],
                                    start=False, stop=True,
                                    tile_position=(0, 32 * jj),
                                )
                            nc.scalar.activation(
                                hc_sb[:, g * OU: (g + 1) * OU], ps_ou[:], AF.Tanh)

                    # ---- final combine: h = hc + r*(state - hc) ----
                    h_sb = lsb2.tile([128, NG * DO], F32, tag="h_sb")
                    nc.vector.tensor_sub(h_sb[:], st_grp[:], hc_sb[:])
                    nc.vector.tensor_mul(
                        h_sb[:].rearrange("p (g o) -> p g o", o=DO),
                        h_sb[:].rearrange("p (g o) -> p g o", o=DO),
                        zr_sb[:].rearrange("p (g o) -> p g o", o=OG)[:, :, DO:OG],
                    )
                    nc.vector.tensor_add(h_sb[:], h_sb[:], hc_sb[:])
                    nc.sync.dma_start(h_out[:], h_sb[:])

    nc.finalize()
    return nc


def _get_nc():
    if "nc" not in _CACHE:
        _CACHE["nc"] = _build()
    return _CACHE["nc"]


def kernel(x, state, node_emb, time_emb, gate_w, gate_b, gate_gamma, gate_beta,
           upd_w, upd_b, upd_gamma, upd_beta):
    global LAST_RESULT
    x = np.asarray(x, np.float32)
    state = np.asarray(state, np.float32)
    node_emb = np.asarray(node_emb, np.float32)
    time_emb = np.asarray(time_emb, np.float32)
    gate_w = np.asarray(gate_w, np.float32)
    gate_b = np.asarray(gate_b, np.float32)
    upd_w = np.asarray(upd_w, np.float32)
    upd_b = np.asarray(upd_b, np.float32)

    shared = (
        np.array_equal(np.asarray(gate_gamma), np.ones(D, np.float32))
        and np.array_equal(np.asarray(upd_gamma), np.ones(D, np.float32))
        and np.array_equal(np.asarray(gate_beta), np.zeros(D, np.float32))
        and np.array_equal(np.asarray(upd_beta), np.zeros(D, np.float32))
    )
    if not shared:
        return _np_reference(x, state, node_emb, time_emb, gate_w, gate_b,
                             gate_gamma, gate_beta, upd_w, upd_b, upd_gamma,
                             upd_beta)

    if os.environ.get("BASS_TRACE"):
        _install_prof_shim()

    from concourse.bass_utils import run_bass_kernel_spmd

    nc = _get_nc()
    in_maps = _prep_inmaps(x, state, node_emb, time_emb, gate_w, gate_b,
                           upd_w, upd_b)

    res = run_bass_kernel_spmd(
        nc, in_maps, list(range(NCORES)),
        trace=bool(os.environ.get("BASS_TRACE")),
    )
    LAST_RESULT = res
    return _unpack_h(lambda c: res.results[c]["h_out"])


def _prep_inmaps(x, state, node_emb, time_emb, gate_w, gate_b, upd_w, upd_b):
    inp = np.concatenate([x, state], -1)                      # [B, N, C]
    inpT = np.ascontiguousarray(inp.transpose(2, 0, 1)).astype(np.float16)
    neT = np.ascontiguousarray(node_emb.T).astype(np.float16)  # [D, N]
    teT = np.ascontiguousarray(time_emb.T).astype(np.float16)  # [D, B]
    pg_h = np.ascontiguousarray(
        gate_w.transpose(1, 0, 3, 2).reshape(128, OG * C)).astype(np.float16)
    pu_h = np.ascontiguousarray(
        upd_w.transpose(1, 0, 3, 2).reshape(128, OU * C)).astype(np.float16)

    in_maps = []
    for c in range(NCORES):
        nsl = slice(c * NL, (c + 1) * NL)
        bsl = slice(c * BL, (c + 1) * BL)
        ne2 = np.empty((128, NL), np.float16)
        ne2[0:64] = neT[:, nsl]
        ne2[64:128] = neT[:, nsl]
        inp_cm = np.ascontiguousarray(
            inp[bsl].reshape(BL, 8, 128, C).transpose(2, 0, 1, 3)
            .reshape(128, BL * 8 * C)).astype(np.float16)
        st_grp = np.ascontiguousarray(
            state[:, nsl, :].reshape(B, NG, 4, DO).transpose(2, 0, 1, 3)
            .reshape(128, NG * DO)).astype(np.float16)
        in_maps.append({
            "neT_full": neT,
            "neT_loc2": ne2,
            "te_col": np.ascontiguousarray(time_emb.T[:, bsl]).astype(np.float32),
            "teT16": teT,
            "gb16": gate_b.astype(np.float16),
            "ub16": upd_b.astype(np.float16),
            "inp_cm_h": inp_cm,
            "inpT_h": np.ascontiguousarray(inpT[:, :, nsl]).reshape(C, B * NL),
            "st_grp_h": st_grp,
            "pg_h": pg_h,
            "pu_h": pu_h,
        })
    return in_maps


def _unpack_h(get_out):
    h = np.empty((B, N, DO), np.float32)
    for c in range(NCORES):
        ho = get_out(c).reshape(4, 32, NG, DO)  # [jj, b, g, o]
        h[:, c * NL: (c + 1) * NL, :] = (
            ho.transpose(1, 2, 0, 3).reshape(B, NL, DO)
        )
    return h


# revision 20
# speedup vs baseline: 1.4791x; 1.4791x over previous
"""Trainium2 Bass kernel for the GRU-GCN cell (nn_GRUCell).

Sharding: 8 NeuronCores.
 - Attention phases (logits/softmax/PV) are BATCH-parallel: each core owns
   4 batches and all 1024 nodes, so matmul free dims are 512 wide and each
   (batch, n-chunk) block is one deep pipeline of 8 matmuls + 8 exps.
 - Weight-gen and the per-node output matmuls are NODE-parallel (128
   nodes/core, all 32 batches): the per-node weights are too large to
   replicate or gather.
 - Three AllToAll collectives redistribute xg2 (gate), z*state, and xg2
   (update) between the batch-parallel and node-parallel layouts.
 - exp(logits) is spilled to DRAM between the gate and update GCNs (the
   update GCN reuses the same softmax).
All matmuls fp16 operands with fp32 PSUM accumulation.
"""

import os
import sys

sys.path.insert(0, "/opt/trn_rl_repo")
import numpy as np

B, N, D = 32, 1024, 64
DI = DO = 64
C = DI + DO          # 128
OG, OU = 2 * DO, DO  # 128, 64
NCORES = 8
NL = N // NCORES     # 128 nodes per core
BL = B // NCORES     # 4 batches per core
NG = NL // 4         # 32 col-pack groups of 4 nodes
EPS = 1e-12

_CACHE = {}
LAST_RESULT = None  # test harness reads timing info from here


def _np_reference(x, state, node_emb, time_emb, gate_w, gate_b, gate_gamma,
                  gate_beta, upd_w, upd_b, upd_gamma, upd_beta):
    """Plain numpy fallback (general layernorm parameters)."""

    def _ln(v, g, b2):
        mu = v.mean(-1, keepdims=True)
        var = ((v - mu) ** 2).mean(-1, keepdims=True)
        return (v - mu) / np.sqrt(var + EPS) * g + b2

    def _gcn(xg, w_pool, b_pool, g, b2):
        emb = _ln(node_emb[None] + time_emb[:, None], g, b2)
        logits = np.einsum("bnd,bmd->bnm", emb, emb, optimize=True)
        a = np.exp(logits - logits.max(-1, keepdims=True))
        a /= a.sum(-1, keepdims=True)
        xg2 = np.einsum("bnm,bmc->bnc", a, xg, optimize=True)
        w = np.einsum("nd,dkio->nkio", node_emb, w_pool, optimize=True)
        bias = time_emb @ b_pool
        return (np.einsum("bni,nio->bno", xg, w[:, 0], optimize=True)
                + np.einsum("bni,nio->bno", xg2, w[:, 1], optimize=True)
                + bias[:, None, :])

    inp = np.concatenate([x, state], -1)
    zr = 1.0 / (1.0 + np.exp(-_gcn(inp, gate_w, gate_b, gate_gamma, gate_beta)))
    z, r = zr[..., :DO], zr[..., DO:]
    cand = np.concatenate([x, z * state], -1)
    hc = np.tanh(_gcn(cand, upd_w, upd_b, upd_gamma, upd_beta))
    return (r * state + (1.0 - r) * hc).astype(np.float32)


def _install_prof_shim():
    """Provide antenv.axon_hooks if absent so trace=True can NTFF-profile."""
    import types

    if "antenv.axon_hooks" in sys.modules:
        return
    try:
        from trn_agent_boot.trn_boot import _ntff_profile_via_ctypes

        hook = _ntff_profile_via_ctypes("/opt/axon/libaxon_pjrt.so")
    except Exception:
        hook = None
    mod = types.ModuleType("antenv.axon_hooks")
    mod.get_axon_ntff_profile_hook = lambda: hook

    def _set(h):
        mod.get_axon_ntff_profile_hook = lambda: h

    mod.set_axon_ntff_profile_hook = _set
    sys.modules["antenv.axon_hooks"] = mod
    try:
        import antenv

        antenv.axon_hooks = mod
    except Exception:
        pass


def _build():
    import concourse.bacc as bacc
    import concourse.mybir as mybir
    from concourse.tile import TileContext
    from concourse.masks import make_identity

    F16 = mybir.dt.float16
    F32 = mybir.dt.float32
    AF = mybir.ActivationFunctionType
    ALU = mybir.AluOpType

    nc = bacc.Bacc()

    def pin(name, shape, dt=F16):
        return nc.declare_dram_parameter(name, shape, dt, isOutput=False)

    neT_full = pin("neT_full", [D, N])          # node_emb^T, all nodes (LN)
    neT_loc2 = pin("neT_loc2", [128, NL])       # local node_emb^T, k-duplicated
    te_col = pin("te_col", [D, BL], F32)             # local time_emb columns
    teT16 = pin("teT16", [D, B])                # bias matmul lhsT
    gb16 = pin("gb16", [D, OG])
    ub16 = pin("ub16", [D, OU])
    inp_cm_h = pin("inp_cm_h", [128, BL * 8 * C])   # [m,(bb,q,c)] PV lhsT
    inpT_h = pin("inpT_h", [C, B * NL])         # c-major local [x;state]
    st_grp_h = pin("st_grp_h", [128, NG * DO])  # grouped local state
    pg_h = pin("pg_h", [128, OG * C])           # gate pool [64k+d,(o,i)]
    pu_h = pin("pu_h", [128, OU * C])           # upd pool
    h_out = nc.declare_dram_parameter("h_out", [128, NG * DO], F32, isOutput=True)

    with TileContext(nc) as tc:
        with (
            tc.tile_pool(name="const", bufs=1) as cpool,
            tc.tile_pool(name="big", bufs=1) as big,
            tc.tile_pool(name="stage", bufs=2) as stg,
            tc.tile_pool(name="dram", bufs=1, space="DRAM") as dram,
            tc.tile_pool(name="pswg", bufs=1, space="PSUM") as pswg,
        ):
            # ---------- constants ----------
            ones16r = cpool.tile([1, 128], F16, tag="ones16r")
            nc.gpsimd.memset(ones16r[:], 1.0)
            ones_col16 = cpool.tile([128, 1], F16, tag="ones_col16")
            nc.gpsimd.memset(ones_col16[:], 1.0)
            ones64 = cpool.tile([D, D], F16, tag="ones64")
            nc.gpsimd.memset(ones64[:], 1.0)
            neg64_col = cpool.tile([128, 1], F32, tag="neg64_col")
            nc.gpsimd.memset(neg64_col[:], -64.0)
            ident16 = cpool.tile([128, 128], F16, tag="ident16")
            make_identity(nc, ident16[:])
            eps_col = cpool.tile([D, 1], F32, tag="eps_col")
            nc.gpsimd.memset(eps_col[:], EPS)

            # ---------- persistent SBUF ----------
            neT_sb = cpool.tile([D, N], F16, tag="neT_sb")
            nc.sync.dma_start(neT_sb[:], neT_full[:])
            neL_sb = cpool.tile([128, NL], F16, tag="neL_sb")
            nc.sync.dma_start(neL_sb[:], neT_loc2[:])
            te_sb = cpool.tile([D, BL], F32, tag="te_sb")
            nc.sync.dma_start(te_sb[:], te_col[:])
            teT_sb = cpool.tile([D, B], F16, tag="teT_sb")
            nc.sync.dma_start(teT_sb[:], teT16[:])
            gb_sb = cpool.tile([D, OG], F16, tag="gb_sb")
            nc.sync.dma_start(gb_sb[:], gb16[:])
            ub_sb = cpool.tile([D, OU], F16, tag="ub_sb")
            nc.sync.dma_start(ub_sb[:], ub16[:])

            wslab = big.tile([C, 2 * OG * NL], F16, tag="wslab")      # 64KB/p
            zr_sb = big.tile([128, NG * OG], F16, tag="zr_sb")
            st_grp = big.tile([128, NG * DO], F16, tag="st_grp")
            nc.sync.dma_start(st_grp[:], st_grp_h[:])
            sinv16 = big.tile([D, BL * N], F16, tag="sinv16")
            biasg_rep = big.tile([128, OG], F32, tag="biasg_rep")
            biasu_rep = big.tile([128, OU], F32, tag="biasu_rep")

            # DRAM scratch: exp spill + three AllToAlls
            d_exp = dram.tile([BL * 2, 128, 8 * 512], F16, tag="d_exp")
            d1a_in = dram.tile([NCORES, C, 2, NL], F16, tag="d1a_in")
            d1b_in = dram.tile([NCORES, C, 2, NL], F16, tag="d1b_in")
            d1a_out = dram.tile([NCORES, C, 2, NL], F16, tag="d1a_out")
            d1b_out = dram.tile([NCORES, C, 2, NL], F16, tag="d1b_out")
            d2_in = dram.tile([NCORES, DO, BL, NL], F16, tag="d2_in")
            d2_out = dram.tile([NCORES, DO, BL, NL], F16, tag="d2_out")
            d3_in = dram.tile([NCORES, DO, BL, NL], F16, tag="d3_in")
            d3_out = dram.tile([NCORES, DO, BL, NL], F16, tag="d3_out")

            # ---------- bias = time_emb @ pool_b, replicated to (jj,b) rows --
            with tc.tile_pool(name="psb", bufs=1, space="PSUM") as psb:
                ps_bg = psb.tile([B, OG], F32, tag="ps_bg")
                nc.tensor.matmul(ps_bg[:], teT_sb[:], gb_sb[:], start=True, stop=True)
                bg_row = stg.tile([B, OG], F32, tag="bg_row")
                nc.vector.tensor_copy(bg_row[:], ps_bg[:])
                ps_bu = psb.tile([B, OU], F32, tag="ps_bu")
                nc.tensor.matmul(ps_bu[:], teT_sb[:], ub_sb[:], start=True, stop=True)
                bu_row = stg.tile([B, OU], F32, tag="bu_row")
                nc.vector.tensor_copy(bu_row[:], ps_bu[:])
                for jj in range(4):
                    nc.gpsimd.dma_start(biasg_rep[32 * jj: 32 * jj + 32, :], bg_row[:])
                    nc.gpsimd.dma_start(biasu_rep[32 * jj: 32 * jj + 32, :], bu_row[:])

            # ================= batch-parallel phase =================
            with tc.tile_pool(name="attn_sb", bufs=1) as asb:
                inp_cm = asb.tile([128, BL * 8 * C], F16, tag="inp_cm")
                nc.sync.dma_start(inp_cm[:], inp_cm_h[:])
                embT = asb.tile([D, BL * N], F16, tag="embT")
                xg2T_loc = asb.tile([C, BL * N], F16, tag="xg2T_loc")

                # ---- layernorm in transposed [d, n] layout, local batches ----
                with (
                    tc.tile_pool(name="ln_sb", bufs=1) as lsb,
                    tc.tile_pool(name="ln_ps", bufs=1, space="PSUM") as lps,
                ):
                    for bb in range(BL):
                        u16 = lsb.tile([D, N], F16, tag="u16")
                        nc.vector.tensor_scalar(
                            out=u16[:], in0=neT_sb[:],
                            scalar1=te_sb[:, bb: bb + 1],
                            scalar2=None, op0=ALU.add,
                        )
                        u2 = lsb.tile([D, N], F16, tag="u2")
                        nc.vector.tensor_mul(u2[:], u16[:], u16[:])
                        ps_sum = lps.tile([D, N], F32, tag="ps_sum")
                        ps_sq = lps.tile([D, N], F32, tag="ps_sq")
                        for hh in range(2):
                            sl = slice(hh * 512, (hh + 1) * 512)
                            nc.tensor.matmul(ps_sum[:, sl], ones64[:], u16[:, sl],
                                             start=True, stop=True)
                            nc.tensor.matmul(ps_sq[:, sl], ones64[:], u2[:, sl],
                                             start=True, stop=True)
                        mu16 = lsb.tile([D, N], F16, tag="mu16")
                        nc.vector.tensor_scalar_mul(mu16[:], ps_sum[:], 1.0 / D)
                        var32 = lsb.tile([D, N], F32, tag="var32")
                        nc.vector.tensor_scalar_mul(var32[:], ps_sq[:], 1.0 / D)
                        musq = lsb.tile([D, N], F32, tag="musq")
                        nc.vector.tensor_mul(musq[:], mu16[:], mu16[:])
                        nc.vector.tensor_sub(var32[:], var32[:], musq[:])
                        nc.scalar.activation(var32[:], var32[:], AF.Sqrt, bias=eps_col[:])
                        nc.vector.reciprocal_approx_fast(var32[:], var32[:])
                        dst = embT[:, bb * N: (bb + 1) * N]
                        nc.vector.tensor_sub(u16[:], u16[:], mu16[:])
                        nc.vector.tensor_mul(dst, u16[:], var32[:])

                # ---- attention: logits -> exp -> s -> PV, per (bb, ncol) ----
                with (
                    tc.tile_pool(name="psl", bufs=3, space="PSUM") as psl,
                    tc.tile_pool(name="psx", bufs=1, space="PSUM") as psx,
                    tc.tile_pool(name="pss", bufs=1, space="PSUM") as pss,
                ):
                    for bb in range(BL):
                        for ncol in range(2):
                            nb = ncol * 512
                            exp_w = stg.tile([128, 8 * 512], F16, tag="exp_w")
                            for q in range(8):
                                ps_l = psl.tile([128, 512], F32, tag="ps_l")
                                nc.tensor.matmul(
                                    ps_l[:],
                                    embT[:, bb * N + q * 128: bb * N + q * 128 + 128],
                                    embT[:, bb * N + nb: bb * N + nb + 512],
                                    start=True, stop=True,
                                )
                                nc.scalar.activation(
                                    exp_w[:, q * 512: (q + 1) * 512],
                                    ps_l[:], AF.Exp, bias=neg64_col[:],
                                )
                            nc.sync.dma_start(d_exp[bb * 2 + ncol], exp_w[:])
                            # denominator: ones-matmul accumulated over m-chunks
                            ps_s = pss.tile([1, 512], F32, tag="ps_s")
                            for q in range(8):
                                nc.tensor.matmul(
                                    ps_s[:], ones_col16[:],
                                    exp_w[:, q * 512: (q + 1) * 512],
                                    start=(q == 0), stop=(q == 7),
                                )
                            s_row = stg.tile([1, 512], F16, tag="s_row")
                            nc.scalar.activation(s_row[:], ps_s[:], AF.Copy)
                            # PV: xg2^T[c, n] accumulated over m-chunks
                            # (normalization deferred to the packed-recip pass)
                            ps_xg2 = psx.tile([C, 512], F32, tag="ps_xg2")
                            for q in range(8):
                                nc.tensor.matmul(
                                    ps_xg2[:],
                                    inp_cm[:, (bb * 8 + q) * C: (bb * 8 + q) * C + C],
                                    exp_w[:, q * 512: (q + 1) * 512],
                                    start=(q == 0), stop=(q == 7),
                                )
                            ps_rep = pss.tile([128, 512], F32, tag="ps_rep")
                            nc.tensor.matmul(ps_rep[:], ones16r[:], s_row[:],
                                             start=True, stop=True)
                            sinv32 = stg.tile([128, 512], F32, tag="sinv32")
                            nc.vector.reciprocal_approx_fast(sinv32[:], ps_rep[:])
                            sl = slice(bb * N + nb, bb * N + nb + 512)
                            nc.vector.tensor_mul(
                                xg2T_loc[:, sl], ps_xg2[:], sinv32[:])
                            nc.scalar.activation(
                                sinv16[:, sl], sinv32[0:D, :], AF.Copy)
                            d1h = d1a_in if bb < 2 else d1b_in
                            nc.gpsimd.dma_start(
                                d1h[:].rearrange("j c bb nn -> c bb j nn")
                                [:, bb % 2, 4 * ncol: 4 * ncol + 4, :],
                                xg2T_loc[:, sl].rearrange(
                                    "c (j nn) -> c j nn", nn=NL),
                            )

                # ---- w-gen gate pool (row-tiled k pairs) ----
                for och in range(8):
                    pwg_t = stg.tile([128, 16 * C], F16, tag="pwg_t")
                    nc.sync.dma_start(pwg_t[:], pg_h[:, och * 16 * C: (och + 1) * 16 * C])
                    for og4 in range(4):
                        ps_wA = pswg.tile([128, 4 * NL], F32, tag="wA")
                        ps_wB = pswg.tile([128, 4 * NL], F32, tag="wB")
                        for oo in range(4):
                            osl = slice((og4 * 4 + oo) * C, (og4 * 4 + oo) * C + C)
                            nc.tensor.matmul(
                                ps_wA[:, oo * NL: (oo + 1) * NL],
                                pwg_t[0:64, osl], neL_sb[0:64, :],
                                start=True, stop=True,
                            )
                            nc.tensor.matmul(
                                ps_wB[:, oo * NL: (oo + 1) * NL],
                                pwg_t[64:128, osl], neL_sb[64:128, :],
                                start=True, stop=True,
                            )
                        ob = (och * 4 + og4) * 4
                        nc.vector.tensor_copy(
                            wslab[:, ob * NL: (ob + 4) * NL], ps_wA[:])
                        nc.scalar.activation(
                            wslab[:, (OG + ob) * NL: (OG + ob + 4) * NL],
                            ps_wB[:], AF.Copy)

                # ---- AllToAll #1: xg2 gate, batch-shard -> node-shard ----

            nc.gpsimd.collective_compute(
                "AllToAll", mybir.AluOpType.bypass,
                replica_groups=[list(range(NCORES))],
                ins=[d1a_in.opt()], outs=[d1a_out.opt()],
            )
            nc.gpsimd.collective_compute(
                "AllToAll", mybir.AluOpType.bypass,
                replica_groups=[list(range(NCORES))],
                ins=[d1b_in.opt()], outs=[d1b_out.opt()],
            )

            # ================= node-parallel phase =================
            with tc.tile_pool(name="mid_sb", bufs=1) as msb:
                inpT_cm = msb.tile([C, B * NL], F16, tag="inpT_cm")
                nc.sync.dma_start(inpT_cm[:], inpT_h[:])
                xg2T_node = msb.tile([C, B * NL], F16, tag="xg2T_node")
                xgnv = xg2T_node[:].rearrange("c (j bb nn) -> c j bb nn",
                                              bb=BL, nn=NL)
                nc.sync.dma_start(
                    xgnv[:, :, 0:2, :].rearrange("c j bb nn -> c j (bb nn)"),
                    d1a_out[:].rearrange("j c bb nn -> c j (bb nn)"),
                )
                nc.sync.dma_start(
                    xgnv[:, :, 2:4, :].rearrange("c j bb nn -> c j (bb nn)"),
                    d1b_out[:].rearrange("j c bb nn -> c j (bb nn)"),
                )

                # ---- gate out-matmuls -> z, r ----
                wview = wslab[:].rearrange("c (k o n) -> c k o n", k=2, o=OG)
                with tc.tile_pool(name="pso", bufs=4, space="PSUM") as pso:
                    for g in range(NG):
                        ps_og = pso.tile([128, OG], F32, tag="og")
                        for jj in range(4):
                            n_ = g * 4 + jj
                            for ki, src in ((0, inpT_cm), (1, xg2T_node)):
                                lhs = src[:].rearrange("c (b n) -> c n b", n=NL)[:, n_, :]
                                rhs = wview[:, ki, :, n_]
                                nc.tensor.matmul(
                                    ps_og[32 * jj: 32 * jj + 32, :],
                                    lhs, rhs,
                                    start=(ki == 0), stop=(ki == 1),
                                    tile_position=(0, 32 * jj),
                                )
                        zt = stg.tile([128, OG], F32, tag="zt")
                        nc.vector.tensor_add(zt[:], ps_og[:], biasg_rep[:])
                        nc.scalar.activation(
                            zr_sb[:, g * OG: (g + 1) * OG], zt[:], AF.Sigmoid)

                with tc.tile_pool(name="late_sb", bufs=1) as lsb2:
                    # keep the A@x channel block + x rows before mid_sb closes
                    xg2u_full = lsb2.tile([C, B * NL], F16, tag="xg2u_full")
                    nc.vector.tensor_copy(xg2u_full[0:DI, :], xg2T_node[0:DI, :])
                    candT = lsb2.tile([C, B * NL], F16, tag="candT")
                    nc.vector.tensor_copy(candT[0:DI, :], inpT_cm[0:DI, :])

                    zs_grp = lsb2.tile([128, NG * DO], F16, tag="zs_grp")
                    nc.vector.tensor_mul(
                        zs_grp[:].rearrange("p (g o) -> p g o", o=DO),
                        zr_sb[:].rearrange("p (g o) -> p g o", o=OG)[:, :, 0:DO],
                        st_grp[:].rearrange("p (g o) -> p g o", o=DO),
                    )

                    # candT rows 64:128 = (z*state)^T local (PE transpose)
                    with tc.tile_pool(name="psz", bufs=2, space="PSUM") as psz:
                        for g in range(NG):
                            ps_zt = psz.tile([DO, 128], F16, tag="ps_zt")
                            nc.tensor.transpose(
                                ps_zt[:], zs_grp[:, g * DO: (g + 1) * DO], ident16[:])
                            dst = (
                                candT[DI:C, :]
                                .rearrange("c (b n) -> c b n", n=NL)
                                [:, :, g * 4: g * 4 + 4]
                            )
                            src = ps_zt[:].rearrange("c (jj b) -> c b jj", jj=4)
                            nc.vector.tensor_copy(dst, src)

                    # ---- AllToAll #2: (z*state)^T, node-shard -> batch-shard
                    zsv = candT[DI:C, :].rearrange("o (b n) -> o b n", n=NL)
                    for j in range(NCORES):
                        nc.gpsimd.dma_start(
                            d2_in[j], zsv[:, BL * j: BL * j + BL, :])
                    nc.gpsimd.collective_compute(
                        "AllToAll", mybir.AluOpType.bypass,
                        replica_groups=[list(range(NCORES))],
                        ins=[d2_in.opt()], outs=[d2_out.opt()],
                    )

                    # ---- w-gen update pool (reuses wslab low half) ----
                    for och in range(4):
                        pwu_t = stg.tile([128, 16 * C], F16, tag="pwg_t")
                        nc.sync.dma_start(
                            pwu_t[:], pu_h[:, och * 16 * C: (och + 1) * 16 * C])
                        for og4 in range(4):
                            ps_wA = pswg.tile([128, 4 * NL], F32, tag="wA")
                            ps_wB = pswg.tile([128, 4 * NL], F32, tag="wB")
                            for oo in range(4):
                                osl = slice((og4 * 4 + oo) * C, (og4 * 4 + oo) * C + C)
                                nc.tensor.matmul(
                                    ps_wA[:, oo * NL: (oo + 1) * NL],
                                    pwu_t[0:64, osl], neL_sb[0:64, :],
                                    start=True, stop=True,
                                )
                                nc.tensor.matmul(
                                    ps_wB[:, oo * NL: (oo + 1) * NL],
                                    pwu_t[64:128, osl], neL_sb[64:128, :],
                                    start=True, stop=True,
                                )
                            ob = (och * 4 + og4) * 4
                            nc.vector.tensor_copy(
                                wslab[:, ob * NL: (ob + 4) * NL], ps_wA[:])
                            nc.scalar.activation(
                                wslab[:, (OU + ob) * NL: (OU + ob + 4) * NL],
                                ps_wB[:], AF.Copy)

                    zs_cmT = lsb2.tile([DO, BL * N], F16, tag="zs_cmT")
                    zcv = zs_cmT[:].rearrange("o (bb j nn) -> o bb j nn",
                                              j=NCORES, nn=NL)
                    for j in range(NCORES):
                        nc.sync.dma_start(zcv[:, :, j, :], d2_out[j])
                    zs_cm = lsb2.tile([128, BL * 8 * DO], F16, tag="zs_cm")
                    with tc.tile_pool(name="pszz", bufs=2, space="PSUM") as pszz:
                        for bb in range(BL):
                            for q in range(8):
                                ps_zc = pszz.tile([128, DO], F16, tag="ps_zc")
                                nc.tensor.transpose(
                                    ps_zc[:],
                                    zs_cmT[:, (bb * 8 + q) * NL:
                                           (bb * 8 + q) * NL + NL],
                                    ident16[0:DO, 0:DO],
                                )
                                nc.vector.tensor_copy(
                                    zs_cm[:, (bb * 8 + q) * DO:
                                          (bb * 8 + q) * DO + DO],
                                    ps_zc[:],
                                )

                    # ---- update PV (z*state part) from spilled exp ----
                    xg2u_loc = lsb2.tile([DO, BL * N], F16, tag="xg2u_loc")
                    with tc.tile_pool(name="psu", bufs=2, space="PSUM") as psu:
                        for bb in range(BL):
                            for ncol in range(2):
                                nb = ncol * 512
                                exp_r = stg.tile([128, 8 * 512], F16, tag="exp_w")
                                nc.sync.dma_start(exp_r[:], d_exp[bb * 2 + ncol])
                                ps_xu = psu.tile([DO, 512], F32, tag="ps_xu")
                                for q in range(8):
                                    nc.tensor.matmul(
                                        ps_xu[:],
                                        zs_cm[:, (bb * 8 + q) * DO:
                                              (bb * 8 + q) * DO + DO],
                                        exp_r[:, q * 512: (q + 1) * 512],
                                        start=(q == 0), stop=(q == 7),
                                    )
                                nc.vector.tensor_mul(
                                    xg2u_loc[:, bb * N + nb: bb * N + nb + 512],
                                    ps_xu[:],
                                    sinv16[:, bb * N + nb: bb * N + nb + 512],
                                )

                    # ---- AllToAll #3: xg2 upd, batch-shard -> node-shard ----
                    xg2uv = xg2u_loc[:].rearrange("c (bb j nn) -> c bb j nn",
                                                  j=NCORES, nn=NL)
                    for j in range(NCORES):
                        nc.gpsimd.dma_start(d3_in[j], xg2uv[:, :, j, :])
                    nc.gpsimd.collective_compute(
                        "AllToAll", mybir.AluOpType.bypass,
                        replica_groups=[list(range(NCORES))],
                        ins=[d3_in.opt()], outs=[d3_out.opt()],
                    )
                    nc.sync.dma_start(
                        xg2u_full[DI:C, :].rearrange("c (j r) -> c j r",
                                                     j=NCORES),
                        d3_out[:].rearrange("j c bb nn -> c j (bb nn)"),
                    )

                    # ---- update out-matmuls -> hc ----
                    hc_sb = lsb2.tile([128, NG * DO], F16, tag="hc_sb")
                    wuview = (
                        wslab[:, : 2 * OU * NL]
                        .rearrange("c (k o n) -> c k o n", k=2, o=OU)
                    )
                    with tc.tile_pool(name="psou", bufs=3, space="PSUM") as psou:
                        for g in range(NG):
                            ps_ou = psou.tile([128, OU], F32, tag="ou")
                            for jj in range(4):
                                n_ = g * 4 + jj
                                for ki, src in ((0, candT), (1, xg2u_full)):
                                    lhs = src[:].rearrange(
                                        "c (b n) -> c n b", n=NL)[:, n_, :]
                                    rhs = wuview[:, ki, :, n_]
                                    nc.tensor.matmul(
                                        ps_ou[32 * jj: 32 * jj + 32, :],
                                        lhs, rhs,
                                        start=(ki == 0), stop=(ki == 1),
                                        tile_position=(0, 32 * jj),
                                    )
                            tt = stg.tile([128, OU], F32, tag="tt")
                            nc.vector.tensor_add(tt[:], ps_ou[:], biasu_rep[:])
                            nc.scalar.activation(
                                hc_sb[:, g * OU: (g + 1) * OU], tt[:], AF.Tanh)

                    # ---- final combine: h = hc + r*(state - hc) ----
                    h_sb = lsb2.tile([128, NG * DO], F32, tag="h_sb")
                    nc.vector.tensor_sub(h_sb[:], st_grp[:], hc_sb[:])
                    nc.vector.tensor_mul(
                        h_sb[:].rearrange("p (g o) -> p g o", o=DO),
                        h_sb[:].rearrange("p (g o) -> p g o", o=DO),
                        zr_sb[:].rearrange("p (g o) -> p g o", o=OG)[:, :, DO:OG],
                    )
                    nc.vector.tensor_add(h_sb[:], h_sb[:], hc_sb[:])
                    nc.sync.dma_start(h_out[:], h_sb[:])

    nc.finalize()
    return nc


def _get_nc():
    if "nc" not in _CACHE:
        _CACHE["nc"] = _build()
    return _CACHE["nc"]


def kernel(x, state, node_emb, time_emb, gate_w, gate_b, gate_gamma, gate_beta,
           upd_w, upd_b, upd_gamma, upd_beta):
    global LAST_RESULT
    x = np.asarray(x, np.float32)
    state = np.asarray(state, np.float32)
    node_emb = np.asarray(node_emb, np.float32)
    time_emb = np.asarray(time_emb, np.float32)
    gate_w = np.asarray(gate_w, np.float32)
    gate_b = np.asarray(gate_b, np.float32)
    upd_w = np.asarray(upd_w, np.float32)
    upd_b = np.asarray(upd_b, np.float32)

    shared = (
        np.array_equal(np.asarray(gate_gamma), np.ones(D, np.float32))
        and np.array_equal(np.asarray(upd_gamma), np.ones(D, np.float32))
        and np.array_equal(np.asarray(gate_beta), np.zeros(D, np.float32))
        and np.array_equal(np.asarray(upd_beta), np.zeros(D, np.float32))
    )
    if not shared:
        return _np_reference(x, state, node_emb, time_emb, gate_w, gate_b,
                             gate_gamma, gate_beta, upd_w, upd_b, upd_gamma,
                             upd_beta)

    if os.environ.get("BASS_TRACE"):
        _install_prof_shim()

    from concourse.bass_utils import run_bass_kernel_spmd

    nc = _get_nc()
    in_maps = _prep_inmaps(x, state, node_emb, time_emb, gate_w, gate_b,
                           upd_w, upd_b)

    res = run_bass_kernel_spmd(
        nc, in_maps, list(range(NCORES)),
        trace=bool(os.environ.get("BASS_TRACE")),
    )
    LAST_RESULT = res
    return _unpack_h(lambda c: res.results[c]["h_out"])


def _prep_inmaps(x, state, node_emb, time_emb, gate_w, gate_b, upd_w, upd_b):
    inp = np.concatenate([x, state], -1)                      # [B, N, C]
    inpT = np.ascontiguousarray(inp.transpose(2, 0, 1)).astype(np.float16)
    neT = np.ascontiguousarray(node_emb.T).astype(np.float16)  # [D, N]
    teT = np.ascontiguousarray(time_emb.T).astype(np.float16)  # [D, B]
    pg_h = np.ascontiguousarray(
        gate_w.transpose(1, 0, 3, 2).reshape(128, OG * C)).astype(np.float16)
    pu_h = np.ascontiguousarray(
        upd_w.transpose(1, 0, 3, 2).reshape(128, OU * C)).astype(np.float16)

    in_maps = []
    for c in range(NCORES):
        nsl = slice(c * NL, (c + 1) * NL)
        bsl = slice(c * BL, (c + 1) * BL)
        ne2 = np.empty((128, NL), np.float16)
        ne2[0:64] = neT[:, nsl]
        ne2[64:128] = neT[:, nsl]
        inp_cm = np.ascontiguousarray(
            inp[bsl].reshape(BL, 8, 128, C).transpose(2, 0, 1, 3)
            .reshape(128, BL * 8 * C)).astype(np.float16)
        st_grp = np.ascontiguousarray(
            state[:, nsl, :].reshape(B, NG, 4, DO).transpose(2, 0, 1, 3)
            .reshape(128, NG * DO)).astype(np.float16)
        in_maps.append({
            "neT_full": neT,
            "neT_loc2": ne2,
            "te_col": np.ascontiguousarray(time_emb.T[:, bsl]).astype(np.float32),
            "teT16": teT,
            "gb16": gate_b.astype(np.float16),
            "ub16": upd_b.astype(np.float16),
            "inp_cm_h": inp_cm,
            "inpT_h": np.ascontiguousarray(inpT[:, :, nsl]).reshape(C, B * NL),
            "st_grp_h": st_grp,
            "pg_h": pg_h,
            "pu_h": pu_h,
        })
    return in_maps


def _unpack_h(get_out):
    h = np.empty((B, N, DO), np.float32)
    for c in range(NCORES):
        ho = get_out(c).reshape(4, 32, NG, DO)  # [jj, b, g, o]
        h[:, c * NL: (c + 1) * NL, :] = (
            ho.transpose(1, 2, 0, 3).reshape(B, NL, DO)
        )
    return h
